# revision 2
# baseline (speedup 1.0000x reference)
"""Trainium2 Bass kernel: multi-head attention (dense transformer block).

Sharding: 8 cores = 4 batches x 2 head-groups (8 heads each).

Per-core dataflow (all operands fp16; PSUM accumulation fp32):
    QT = Wq_hg^T @ x^T        [512 f, 2048 n]   (features on partitions)
    KT = Wk_hg^T @ x^T        [512 f, 2048 n]
    V  = x @ Wv_hg            [2048 j, 8 h, 64+1]  (ones column -> softmax Z)
    scores: per (head, j-block, i-chunk): S^T = K_h^T.T @ Q_h^T  [128 j, 512 i]
    exp on ScalarE over [128, 2048]-free granules -> P^T fp16 in SBUF
    AV (transposed): per (head, i-block): O_acc[128 i, 65] = sum_j P^T.T @ [V_h | 1]
    normalize on DVE: O = O_acc[:, :64] * (1/O_acc[:, 64])  -> O [n, f] fp16
    O^T via DMA-xbar transpose (SBUF->SBUF) -> OT [f, n]
    out = OT.T @ Wo  [2048 n, 1024 e] -> DRAM
Host: out[b] = partial[2b] + partial[2b+1] + b_out.

The head loop is software-pipelined: scores/exp of head h interleave with the
AV matmuls of head h-1 on the PE while ScalarE (the bottleneck: 33.5M exps
per core) stays saturated; QK-projection f-blocks 2-3 and the output
projection of the previous i-quarter fill residual PE slack.
"""

import os
import numpy as np

os.environ.setdefault("MYCRO_LOCAL_CACHE", "1")

DIM = 1024
HEADS = 16
DIM_HEAD = 64
INNER = HEADS * DIM_HEAD      # 1024
SEQ = 2048
BATCH = 4
NCORES = 8
HG = 2                        # tensor-parallel head groups
NH = HEADS // HG              # 8 heads per core
HG_F = NH * DIM_HEAD          # 512 local inner features
SCALE = DIM_HEAD ** -0.5      # 1/8

DCS = DIM // 128              # 8 contraction chunks for projections
NTS = SEQ // 512              # 4 n-chunks for QK-proj
FBS = HG_F // 128             # 4 feature blocks of QT/KT (2 heads each)
JBS = SEQ // 128              # 16 key blocks
NIQ = 4                       # i-quarters (512 queries each)
NGR = 4                       # score granules per (head, iq): 4 j-blocks each
IBQ = 4                       # i-128-blocks per i-quarter
NBS = SEQ // 128              # 16 n-blocks overall
EBS = DIM // 512              # 2 output column chunks

_STATE = None


def _build_module():
    from contextlib import ExitStack
    import concourse.bacc as bacc
    import concourse.tile as tile
    import concourse.mybir as mybir

    f32 = mybir.dt.float32
    f16 = mybir.dt.float16
    Exp = mybir.ActivationFunctionType.Exp
    Copy = mybir.ActivationFunctionType.Copy

    nc = bacc.Bacc("TRN2", target_bir_lowering=False, debug=False,
                   num_devices=NCORES)

    xt_d = nc.dram_tensor("xt", [DIM, SEQ], f16, kind="ExternalInput").ap()
    wq_d = nc.dram_tensor("wq", [DIM, HG_F], f16, kind="ExternalInput").ap()
    wk_d = nc.dram_tensor("wk", [DIM, HG_F], f16, kind="ExternalInput").ap()
    wv_d = nc.dram_tensor("wv", [DIM, HG_F], f16, kind="ExternalInput").ap()
    wo_d = nc.dram_tensor("wo", [HG_F, DIM], f16, kind="ExternalInput").ap()
    out_d = nc.dram_tensor("out", [SEQ, DIM], f32, kind="ExternalOutput").ap()

    with tile.TileContext(nc) as tc, ExitStack() as ctx:
        # --- PSUM pools: 4 (scores) + 2 (AV) + 2 (proj/qkv) = 8 banks ---
        ps_s_pool = ctx.enter_context(
            tc.tile_pool(name="pss", bufs=1, space="PSUM"))
        ps_av_pool = ctx.enter_context(
            tc.tile_pool(name="psav", bufs=2, space="PSUM"))
        ps_acc_pool = ctx.enter_context(
            tc.tile_pool(name="psacc", bufs=2, space="PSUM"))

        qt_pool = ctx.enter_context(tc.tile_pool(name="qtp", bufs=FBS))
        kt_pool = ctx.enter_context(tc.tile_pool(name="ktp", bufs=FBS))
        v_pool = ctx.enter_context(tc.tile_pool(name="vpp", bufs=JBS))
        pt_pool = ctx.enter_context(tc.tile_pool(name="ptp", bufs=12))
        o_pool = ctx.enter_context(tc.tile_pool(name="opp", bufs=NBS))
        ot_pool = ctx.enter_context(tc.tile_pool(name="otp", bufs=FBS))
        wo_pool = ctx.enter_context(tc.tile_pool(name="wop", bufs=FBS))
        outs_pool = ctx.enter_context(tc.tile_pool(name="outs", bufs=3))
        rc_pool = ctx.enter_context(tc.tile_pool(name="rcp", bufs=4))

        qt = [qt_pool.tile([128, SEQ], f16, tag="qt", name=f"qt{i}")
              for i in range(FBS)]
        kt = [kt_pool.tile([128, SEQ], f16, tag="kt", name=f"kt{i}")
              for i in range(FBS)]
        vp = [v_pool.tile([128, NH, DIM_HEAD + 1], f16, tag="vp",
                          name=f"vp{i}") for i in range(JBS)]
        ob = [o_pool.tile([128, HG_F], f16, tag="ob", name=f"ob{i}")
              for i in range(NBS)]
        ot = [ot_pool.tile([128, SEQ], f16, tag="ot", name=f"ot{i}")
              for i in range(FBS)]
        wo = [wo_pool.tile([128, DIM], f16, tag="wo", name=f"wo{i}")
              for i in range(FBS)]
        for fc in range(FBS):
            nc.sync.dma_start(wo[fc][:, :], wo_d[fc * 128:(fc + 1) * 128, :])

        with tc.tile_pool(name="xtp", bufs=DCS) as xt_pool, \
             tc.tile_pool(name="wqp", bufs=DCS) as wq_pool, \
             tc.tile_pool(name="wkp", bufs=DCS) as wk_pool, \
             tc.tile_pool(name="wvp", bufs=DCS) as wv_pool:

            xt, wqt, wkt, wvt = [], [], [], []
            for dc in range(DCS):
                t = xt_pool.tile([128, SEQ], f16, tag="xt", name=f"xt{dc}")
                nc.sync.dma_start(t[:, :], xt_d[dc * 128:(dc + 1) * 128, :])
                xt.append(t)
            for (pool, dst, src, w) in ((wv_pool, wvt, wv_d, "wv"),
                                        (wq_pool, wqt, wq_d, "wq"),
                                        (wk_pool, wkt, wk_d, "wk")):
                for dc in range(DCS):
                    t = pool.tile([128, HG_F], f16, tag=w, name=f"{w}{dc}")
                    nc.sync.dma_start(t[:, :], src[dc * 128:(dc + 1) * 128, :])
                    dst.append(t)

            def emit_v(nb):
                ps = ps_acc_pool.tile([128, NH, DIM_HEAD], f32, tag="acc")
                for dc in range(DCS):
                    nc.tensor.matmul(
                        ps[:, :, :],
                        xt[dc][:, nb * 128:(nb + 1) * 128],
                        wvt[dc][:, :],
                        start=(dc == 0), stop=(dc == DCS - 1))
                nc.scalar.activation(vp[nb][:, :, 0:DIM_HEAD], ps[:, :, :],
                                     Copy)
                nc.gpsimd.memset(vp[nb][:, :, DIM_HEAD:DIM_HEAD + 1], 1.0)

            def emit_fb(fb, w, dst):
                # one projection f-block (128 features = 2 heads) of Q or K
                for nt in range(NTS):
                    ps = ps_acc_pool.tile([128, 512], f32, tag="acc")
                    for dc in range(DCS):
                        nc.tensor.matmul(
                            ps[:, :],
                            w[dc][:, fb * 128:(fb + 1) * 128],
                            xt[dc][:, nt * 512:(nt + 1) * 512],
                            start=(dc == 0), stop=(dc == DCS - 1))
                    nc.scalar.activation(
                        dst[fb][:, nt * 512:(nt + 1) * 512], ps[:, :], Copy)

            # ---- prologue: V, QK f-blocks 0-1 ----
            for nb in range(JBS):
                emit_v(nb)
            for fb in (0, 1):
                emit_fb(fb, wqt, qt)
                emit_fb(fb, wkt, kt)

            # ---- attention, software-pipelined over (iq, h) ----
            pt_tiles = {}   # (iq, h) -> list of 4 granule tiles

            def emit_scores_granule(iq, h, g):
                fb, p0 = h // 2, (h % 2) * 64
                ps = ps_s_pool.tile([128, NGR, 512], f32, tag="sc")
                for j2 in range(NGR):
                    jb = g * NGR + j2
                    nc.tensor.matmul(
                        ps[:, j2, :],
                        kt[fb][p0:p0 + 64, jb * 128:(jb + 1) * 128],
                        qt[fb][p0:p0 + 64, iq * 512:(iq + 1) * 512],
                        start=True, stop=True)
                pt = pt_pool.tile([128, NGR, 512], f16, tag="pt",
                                  name=f"pt{iq}_{h}_{g}")
                nc.scalar.activation(pt[:, :, :], ps[:, :, :], Exp,
                                     scale=SCALE)
                pt_tiles[(iq, h)].append(pt)

            def emit_av(iq, h, ib):
                pts = pt_tiles[(iq, h)]
                acc = ps_av_pool.tile([128, DIM_HEAD + 1], f32, tag="av")
                for jb in range(JBS):
                    nc.tensor.matmul(
                        acc[:, :],
                        pts[jb // NGR][:, jb % NGR, ib * 128:(ib + 1) * 128],
                        vp[jb][:, h, :],
                        start=(jb == 0), stop=(jb == JBS - 1))
                rc = rc_pool.tile([128, 1], f32, tag="rc")
                with nc.allow_low_precision("softmax recip"):
                    nc.vector.reciprocal(rc[:, :],
                                         acc[:, DIM_HEAD:DIM_HEAD + 1])
                nc.vector.tensor_scalar(
                    ob[iq * IBQ + ib][:, h * DIM_HEAD:(h + 1) * DIM_HEAD],
                    acc[:, 0:DIM_HEAD], rc[:, :], None,
                    mybir.AluOpType.mult)

            def emit_trans(iq):
                for ib in range(IBQ):
                    nbg = iq * IBQ + ib
                    for fb in range(FBS):
                        nc.sync.dma_start_transpose(
                            ot[fb][:, nbg * 128:(nbg + 1) * 128],
                            ob[nbg][:, fb * 128:(fb + 1) * 128])

            def emit_proj(iq, ib):
                nbg = iq * IBQ + ib
                for eb in range(EBS):
                    ps = ps_acc_pool.tile([128, 512], f32, tag="acc")
                    for fc in range(FBS):
                        nc.tensor.matmul(
                            ps[:, :],
                            ot[fc][:, nbg * 128:(nbg + 1) * 128],
                            wo[fc][:, eb * 512:(eb + 1) * 512],
                            start=(fc == 0), stop=(fc == FBS - 1))
                    os_ = outs_pool.tile([128, 512], f32, tag="os")
                    nc.vector.tensor_copy(os_[:, :], ps[:, :])
                    nc.sync.dma_start(
                        out_d[nbg * 128:(nbg + 1) * 128,
                              eb * 512:(eb + 1) * 512],
                        os_[:, :])

            # filler schedule: (iq, h) -> list of thunks emitted after that
            # head's score granules
            fillers = {}
            fillers[(0, 0)] = [lambda: emit_fb(2, wqt, qt)]
            fillers[(0, 1)] = [lambda: emit_fb(2, wkt, kt)]
            fillers[(0, 2)] = [lambda: emit_fb(3, wqt, qt)]
            fillers[(0, 3)] = [lambda: emit_fb(3, wkt, kt)]
            for iq in range(1, NIQ):
                fillers[(iq, 1)] = [lambda iq=iq: emit_trans(iq - 1)]
                for ib in range(IBQ):
                    fillers[(iq, 3 + ib)] = [
                        lambda iq=iq, ib=ib: emit_proj(iq - 1, ib)]

            seq = [(iq, h) for iq in range(NIQ) for h in range(NH)]
            for idx, (iq, h) in enumerate(seq):
                pt_tiles[(iq, h)] = []
                prev = seq[idx - 1] if idx > 0 else None
                for g in range(NGR):
                    emit_scores_granule(iq, h, g)
                    if prev is not None:
                        emit_av(prev[0], prev[1], g)
                if prev is not None:
                    del pt_tiles[prev]
                for thunk in fillers.get((iq, h), ()):
                    thunk()

            # tail: AV of the last head, then last quarter's projection
            for g in range(IBQ):
                emit_av(NIQ - 1, NH - 1, g)
            emit_trans(NIQ - 1)
            for ib in range(IBQ):
                emit_proj(NIQ - 1, ib)

    nc.compile()
    return nc


def _make_in_maps(x, w_qkv, w_out):
    xt = np.ascontiguousarray(
        np.asarray(x, np.float32).transpose(0, 2, 1)).astype(np.float16)
    w_qkv = np.asarray(w_qkv, np.float32)
    w_out = np.asarray(w_out, np.float32)
    in_maps = []
    for c in range(NCORES):
        b, hg = divmod(c, HG)
        f0 = hg * HG_F
        in_maps.append({
            "xt": xt[b],
            "wq": np.ascontiguousarray(
                w_qkv[:, f0:f0 + HG_F]).astype(np.float16),
            "wk": np.ascontiguousarray(
                w_qkv[:, INNER + f0:INNER + f0 + HG_F]).astype(np.float16),
            "wv": np.ascontiguousarray(
                w_qkv[:, 2 * INNER + f0:2 * INNER + f0 + HG_F]).astype(
                    np.float16),
            "wo": np.ascontiguousarray(
                w_out[f0:f0 + HG_F, :]).astype(np.float16),
        })
    return in_maps


def run(x, w_qkv, w_out, **spmd_kwargs):
    """Build (once) + execute on 8 cores; returns BassKernelResults."""
    global _STATE
    from concourse.bass_utils import run_bass_kernel_spmd
    if _STATE is None:
        _STATE = _build_module()
    in_maps = _make_in_maps(x, w_qkv, w_out)
    return run_bass_kernel_spmd(_STATE, in_maps,
                                core_ids=list(range(NCORES)), **spmd_kwargs)


def kernel(x, w_qkv, w_out, b_out):
    res = run(x, w_qkv, w_out)
    parts = [np.asarray(res.results[c]["out"]) for c in range(NCORES)]
    b_out = np.asarray(b_out, np.float32)
    out = np.stack([parts[HG * b] + parts[HG * b + 1] for b in range(BATCH)])
    out += b_out[None, None, :]
    return out.astype(np.float32, copy=False)


# revision 46
# speedup vs baseline: 1.4699x; 1.4699x over previous
"""Trainium2 Bass kernel: multi-head attention (dense transformer block).

Sharding: 8 cores = 4 batches x 2 head-groups (8 heads each).

Per-core dataflow (all operands fp16; PSUM accumulation fp32):
    QT = Wq_hg^T @ x^T        [512 f, 2048 n]   (features on partitions)
    KT = Wk_hg^T @ x^T        [512 f, 2048 n]
    V  = x @ Wv_hg            [2048 j, 8 h, 64+1]  (ones column -> softmax Z)
    scores: per (head, j-block, i-chunk): S^T = K_h^T.T @ Q_h^T  [128 j, N i]
    exp on ScalarE over [128, 1024]-free granules -> P^T fp16 in SBUF
    AV (transposed): per (head, i-block): O_acc[128 i, 65] = sum_j P^T.T @ [V_h | 1]
    normalize on DVE: O = O_acc[:, :64] * (1/O_acc[:, 64])  -> O [n, f] fp16
    O^T via DMA-xbar transpose (SBUF->SBUF) -> OT [f, n]
    out = OT.T @ Wo  [2048 n, 1024 e] -> DRAM
Host: out[b] = partial[2b] + partial[2b+1] + b_out.

Schedule: ScalarE (33.5M exps/core, ~266us) is the bottleneck; everything
else is slotted around keeping it saturated. The i-range is processed in
three phases [1024, 512, 512]; the first is wide so the exp stream can hide
the QK/V projections, which are emitted as ~1.7us filler units placed just
before their consumer head needs them. AV of head h runs one slot behind its
exp; the output projection of each phase fills the next phase's PE slack.
"""

import os
import numpy as np

os.environ.setdefault("MYCRO_LOCAL_CACHE", "1")

DIM = 1024
HEADS = 16
DIM_HEAD = 64
INNER = HEADS * DIM_HEAD      # 1024
SEQ = 2048
BATCH = 4
NCORES = 8
HG = 2                        # tensor-parallel head groups
NH = HEADS // HG              # 8 heads per core
HG_F = NH * DIM_HEAD          # 512 local inner features
SCALE = DIM_HEAD ** -0.5      # 1/8

DCS = DIM // 128              # 8 contraction chunks for projections
NTS = SEQ // 512              # 4 n-chunks for QK-proj
FBS = HG_F // 128             # 4 feature blocks of QT/KT (2 heads each)
JBS = SEQ // 128              # 16 key blocks
EBS = DIM // 512              # 2 output column chunks

# i-range phases: (i0, width). Phase 0 is wide so the exp stream can hide
# the projection prologue; the last phases are narrow so the serial tail
# (AV of the last head -> transpose -> projection -> DMA) is short.
IQS = [(0, 1024), (1024, 512), (1536, 512)]

_STATE = None


def _build_module():
    from contextlib import ExitStack
    import concourse.bacc as bacc
    import concourse.tile as tile
    import concourse.mybir as mybir

    f32 = mybir.dt.float32
    f16 = mybir.dt.float16
    Exp = mybir.ActivationFunctionType.Exp
    Copy = mybir.ActivationFunctionType.Copy

    nc = bacc.Bacc("TRN2", target_bir_lowering=False, debug=False,
                   num_devices=NCORES)

    xt_d = nc.dram_tensor("xt", [DIM, SEQ], f16, kind="ExternalInput").ap()
    wq_d = nc.dram_tensor("wq", [DIM, HG_F], f16, kind="ExternalInput").ap()
    wk_d = nc.dram_tensor("wk", [DIM, HG_F], f16, kind="ExternalInput").ap()
    wv_d = nc.dram_tensor("wv", [DIM, HG_F], f16, kind="ExternalInput").ap()
    wo_d = nc.dram_tensor("wo", [HG_F, DIM], f16, kind="ExternalInput").ap()
    out_d = nc.dram_tensor("out", [SEQ, DIM], f32, kind="ExternalOutput").ap()

    with tile.TileContext(nc) as tc, ExitStack() as ctx:
        # --- PSUM pools: 2x2 (scores) + 2 (AV) + 2 (proj/qkv) = 8 banks ---
        ps_s_pool = ctx.enter_context(
            tc.tile_pool(name="pss", bufs=2, space="PSUM"))
        ps_av_pool = ctx.enter_context(
            tc.tile_pool(name="psav", bufs=2, space="PSUM"))
        ps_acc_pool = ctx.enter_context(
            tc.tile_pool(name="psacc", bufs=2, space="PSUM"))

        # SBUF tile slots are 4KB-per-partition quanta; small tiles are
        # packed into 4KB-aligned groups to avoid 4x padding waste.
        qt_pool = ctx.enter_context(tc.tile_pool(name="qtp", bufs=FBS))
        kt_pool = ctx.enter_context(tc.tile_pool(name="ktp", bufs=FBS))
        v_pool = ctx.enter_context(tc.tile_pool(name="vpp", bufs=1))
        pt_pool = ctx.enter_context(tc.tile_pool(name="ptp", bufs=12))
        rc_pool = ctx.enter_context(tc.tile_pool(name="rcp", bufs=1))
        o_pool = ctx.enter_context(tc.tile_pool(name="opp", bufs=4))
        ot_pool = ctx.enter_context(tc.tile_pool(name="otp", bufs=FBS))
        wo_pool = ctx.enter_context(tc.tile_pool(name="wop", bufs=2))
        outs_pool = ctx.enter_context(tc.tile_pool(name="outs", bufs=2))

        qt = [qt_pool.tile([128, SEQ], f16, tag="qt", name=f"qt{i}")
              for i in range(FBS)]
        kt = [kt_pool.tile([128, SEQ], f16, tag="kt", name=f"kt{i}")
              for i in range(FBS)]
        vpt = v_pool.tile([128, JBS, NH, DIM_HEAD + 1], f16, tag="vp",
                          name="vpt")
        obt = [o_pool.tile([128, 4, HG_F], f16, tag="ob", name=f"ob{i}")
               for i in range(4)]

        def ob_ap(nbg):
            return obt[nbg // 4][:, nbg % 4, :]
        ot = [ot_pool.tile([128, SEQ], f16, tag="ot", name=f"ot{i}")
              for i in range(FBS)]
        wot = [wo_pool.tile([128, 2, DIM], f16, tag="wo", name=f"wo{i}")
               for i in range(2)]
        rct = rc_pool.tile([128, 16], f32, tag="rc", name="rct")
        nc.vector.memset(vpt[:, :, :, DIM_HEAD:DIM_HEAD + 1], 1.0)

        with tc.tile_pool(name="xtp", bufs=DCS // 2) as xt_pool, \
             tc.tile_pool(name="wqp", bufs=2) as wq_pool, \
             tc.tile_pool(name="wkp", bufs=2) as wk_pool, \
             tc.tile_pool(name="wvp", bufs=2) as wv_pool:

            # xt split into two n-half tile-packs per pair of 128-row
            # chunks so the first projection matmuls can start after ~half
            # the input DMA. [128, 2, 1024]: dim1 = dc parity.
            xtp = [[xt_pool.tile([128, 2, 1024], f16, tag=f"xt{hf}",
                                 name=f"xt{p}_{hf}") for p in range(DCS // 2)]
                   for hf in range(2)]
            wvp = [wv_pool.tile([128, 4, HG_F], f16, tag="wv",
                                name=f"wv{p}") for p in range(2)]
            wqp = [wq_pool.tile([128, 4, HG_F], f16, tag="wq",
                                name=f"wq{p}") for p in range(2)]
            wkp = [wk_pool.tile([128, 4, HG_F], f16, tag="wk",
                                name=f"wk{p}") for p in range(2)]
            for dc in range(DCS):
                nc.sync.dma_start(wvp[dc // 4][:, dc % 4, :],
                                  wv_d[dc * 128:(dc + 1) * 128, :])
            for dc in range(DCS):
                nc.sync.dma_start(xtp[0][dc // 2][:, dc % 2, :],
                                  xt_d[dc * 128:(dc + 1) * 128, 0:1024])
            for dc in range(DCS):
                nc.sync.dma_start(xtp[1][dc // 2][:, dc % 2, :],
                                  xt_d[dc * 128:(dc + 1) * 128, 1024:2048])
            for dc in range(DCS):
                nc.sync.dma_start(wkp[dc // 4][:, dc % 4, :],
                                  wk_d[dc * 128:(dc + 1) * 128, :])
            for dc in range(DCS):
                nc.sync.dma_start(wqp[dc // 4][:, dc % 4, :],
                                  wq_d[dc * 128:(dc + 1) * 128, :])
            for fc in range(FBS):
                nc.sync.dma_start(wot[fc // 2][:, fc % 2, :],
                                  wo_d[fc * 128:(fc + 1) * 128, :])

            def xt_ap(dc, n0, n1):
                hf = n0 // 1024
                assert n1 <= (hf + 1) * 1024
                return xtp[hf][dc // 2][:, dc % 2,
                                        n0 - hf * 1024:n1 - hf * 1024]

            def emit_v(nb):
                ps = ps_acc_pool.tile([128, NH, DIM_HEAD], f32, tag="acc")
                for dc in range(DCS):
                    nc.tensor.matmul(
                        ps[:, :, :],
                        xt_ap(dc, nb * 128, (nb + 1) * 128),
                        wvp[dc // 4][:, dc % 4, :],
                        start=(dc == 0), stop=(dc == DCS - 1))
                nc.vector.tensor_copy(vpt[:, nb, :, 0:DIM_HEAD], ps[:, :, :])

            def emit_fb_nt(fb, w, dst, nt, on_dve=False):
                # one 512-n chunk of one projection f-block of Q or K
                ps = ps_acc_pool.tile([128, 512], f32, tag="acc")
                for dc in range(DCS):
                    nc.tensor.matmul(
                        ps[:, :],
                        w[dc // 4][:, dc % 4, fb * 128:(fb + 1) * 128],
                        xt_ap(dc, nt * 512, (nt + 1) * 512),
                        start=(dc == 0), stop=(dc == DCS - 1))
                dst_ap = dst[fb][:, nt * 512:(nt + 1) * 512]
                if on_dve:
                    nc.vector.tensor_copy(dst_ap, ps[:, :])
                else:
                    nc.scalar.activation(dst_ap, ps[:, :], Copy)

            # ---- prologue: V (AV of head 0 needs all of it one slot into
            # the attention stream), kt[0] and the first two qt[0] chunks ----
            for nb in range(12):
                emit_v(nb)
            emit_fb_nt(0, wkp, kt, 0)
            emit_fb_nt(0, wkp, kt, 1)
            emit_fb_nt(0, wqp, qt, 0)
            emit_fb_nt(0, wqp, qt, 1)
            emit_fb_nt(0, wkp, kt, 2)
            emit_fb_nt(0, wkp, kt, 3)

            # ---- attention, software-pipelined over (phase, head) ----
            pt_tiles = {}   # (t, h) -> list of granule tiles

            def emit_scores_granule(t, h, g):
                i0, w = IQS[t]
                fb, p0 = h // 2, (h % 2) * 64
                packs = pt_tiles[(t, h)]
                if w == 1024:
                    jb = g
                    ps = ps_s_pool.tile([128, 1, 1024], f32, tag="sc")
                    for ic in range(2):
                        nc.tensor.matmul(
                            ps[:, 0, ic * 512:(ic + 1) * 512],
                            kt[fb][p0:p0 + 64, jb * 128:(jb + 1) * 128],
                            qt[fb][p0:p0 + 64,
                                   i0 + ic * 512:i0 + (ic + 1) * 512],
                            start=True, stop=True)
                    if g % 2 == 0:
                        packs.append(pt_pool.tile(
                            [128, 2, 1024], f16, tag="pt",
                            name=f"pt{t}_{h}_{g // 2}"))
                    dst = packs[-1][:, g % 2:g % 2 + 1, :]
                else:
                    jpg = 1024 // w          # j-blocks per granule (2 or 4)
                    ps = ps_s_pool.tile([128, jpg, w], f32, tag="sc")
                    for j2 in range(jpg):
                        jb = g * jpg + j2
                        nc.tensor.matmul(
                            ps[:, j2, :],
                            kt[fb][p0:p0 + 64, jb * 128:(jb + 1) * 128],
                            qt[fb][p0:p0 + 64, i0:i0 + w],
                            start=True, stop=True)
                    if g % 2 == 0:
                        packs.append(pt_pool.tile(
                            [128, 2 * jpg, w], f16, tag="pt",
                            name=f"pt{t}_{h}_{g // 2}"))
                    dst = packs[-1][:, (g % 2) * jpg:(g % 2 + 1) * jpg, :]
                nc.scalar.activation(dst, ps[:, :, :], Exp, scale=SCALE)

            av_n = [0]

            def emit_av(t, h, ib):
                i0, w = IQS[t]
                pts = pt_tiles[(t, h)]
                acc = ps_av_pool.tile([128, DIM_HEAD + 1], f32, tag="av")
                jpp = max(2, 2048 // w)      # j-blocks per pt pack
                for jb in range(JBS):
                    lhs = pts[jb // jpp][:, jb % jpp,
                              ib * 128:(ib + 1) * 128]
                    nc.tensor.matmul(
                        acc[:, :], lhs, vpt[:, jb, h, :],
                        start=(jb == 0), stop=(jb == JBS - 1))
                rc = rct[:, av_n[0] % 16:av_n[0] % 16 + 1]
                av_n[0] += 1
                with nc.allow_low_precision("softmax recip"):
                    nc.vector.reciprocal(rc, acc[:, DIM_HEAD:DIM_HEAD + 1])
                nbg = i0 // 128 + ib
                nc.vector.tensor_scalar(
                    ob_ap(nbg)[:, h * DIM_HEAD:(h + 1) * DIM_HEAD],
                    acc[:, 0:DIM_HEAD], rc, None,
                    mybir.AluOpType.mult)

            def emit_trans_fb(t, fb):
                i0, w = IQS[t]
                for ib in range(w // 128):
                    nbg = i0 // 128 + ib
                    nc.sync.dma_start_transpose(
                        ot[fb][:, nbg * 128:(nbg + 1) * 128],
                        ob_ap(nbg)[:, fb * 128:(fb + 1) * 128])

            def emit_trans(t):
                for fb in range(FBS):
                    emit_trans_fb(t, fb)

            def emit_proj(t, ib):
                nbg = IQS[t][0] // 128 + ib
                os_ = outs_pool.tile([128, EBS, 512], f32, tag="os")
                for eb in range(EBS):
                    ps = ps_acc_pool.tile([128, 512], f32, tag="acc")
                    for fc in range(FBS):
                        nc.tensor.matmul(
                            ps[:, :],
                            ot[fc][:, nbg * 128:(nbg + 1) * 128],
                            wot[fc // 2][:, fc % 2,
                                         eb * 512:(eb + 1) * 512],
                            start=(fc == 0), stop=(fc == FBS - 1))
                    nc.vector.tensor_copy(os_[:, eb, :], ps[:, :])
                    nc.sync.dma_start(
                        out_d[nbg * 128:(nbg + 1) * 128,
                              eb * 512:(eb + 1) * 512],
                        os_[:, eb, :])

            def u_fb(fb, w, dst, nt):
                return lambda: emit_fb_nt(fb, w, dst, nt, on_dve=True)

            def u_av(i, ib):
                return lambda: emit_av(seq[i][0], seq[i][1], ib)

            def u_v(nb):
                return lambda: emit_v(nb)

            def u_proj(t, ib):
                return lambda: emit_proj(t, ib)

            seq = [(t, h) for t in range(len(IQS)) for h in range(NH)]
            NSLOT = len(seq)

            # AV of head-slot i runs one slot later (pt pool holds two head
            # windows of the wide phase).
            av_due = {}
            for i in range(NSLOT):
                av_due.setdefault(i + 1, []).extend(
                    u_av(i, ib) for ib in range(IQS[seq[i][0]][1] // 128))

            # Projection filler units, placed just before their consumer.
            fill = {
                (0, 0): [u_v(12), u_v(13), u_v(14), u_v(15)],
                (0, 1): [u_fb(1, wkp, kt, 0), u_fb(1, wkp, kt, 1),
                         u_fb(1, wkp, kt, 2), u_fb(1, wkp, kt, 3),
                         u_fb(1, wqp, qt, 0), u_fb(1, wqp, qt, 1)],
                (0, 2): [u_fb(2, wkp, kt, 0), u_fb(2, wkp, kt, 1)],
                (0, 3): [u_fb(2, wkp, kt, 2), u_fb(2, wkp, kt, 3),
                         u_fb(2, wqp, qt, 0), u_fb(2, wqp, qt, 1)],
                (0, 4): [u_fb(3, wkp, kt, 0), u_fb(3, wkp, kt, 1)],
                (0, 5): [u_fb(3, wkp, kt, 2), u_fb(3, wkp, kt, 3),
                         u_fb(3, wqp, qt, 0), u_fb(3, wqp, qt, 1)],
                (0, 6): [u_fb(0, wqp, qt, 2), u_fb(0, wqp, qt, 3)],
                (0, 7): [u_fb(1, wqp, qt, 2), u_fb(1, wqp, qt, 3)],
                (1, 0): [u_fb(2, wqp, qt, 2), u_fb(2, wqp, qt, 3)],
                (1, 1): [u_fb(3, wqp, qt, 2), u_fb(3, wqp, qt, 3)],
            }
            # output projection of phase t fills slots of phase t+1 (after
            # its transposes, which follow the AV of each head pair)
            for h in range(2, NH):
                fill.setdefault((1, h), []).append(u_proj(0, h - 2))
            fill.setdefault((2, 0), []).append(u_proj(0, 6))
            fill.setdefault((2, 1), []).append(u_proj(0, 7))
            for ib in range(4):
                fill.setdefault((2, 2 + ib), []).append(u_proj(1, ib))

            # transpose of O columns for head pair (2fb, 2fb+1) of phase t
            # becomes ready one slot after AV(t, 2fb+1), i.e. at slot index
            # t*NH + 2fb + 3; the fb=3 pair rolls into the next phase.
            trans_due = {}
            for t in range(len(IQS)):
                for fb in range(FBS):
                    trans_due.setdefault(t * NH + 2 * fb + 3, []).append(
                        (t, fb))

            for idx, (t, h) in enumerate(seq):
                pt_tiles[(t, h)] = []
                ngr = IQS[t][1] // 64
                for (tt, fb) in trans_due.get(idx, ()):
                    emit_trans_fb(tt, fb)   # DMA-engine work, no PE cost
                items = av_due.get(idx, []) + list(fill.get((t, h), ()))
                for g in range(ngr):
                    emit_scores_granule(t, h, g)
                    if g < len(items):
                        items[g]()
                for it in items[ngr:]:
                    it()
                if idx >= 1:
                    pt_tiles.pop(seq[idx - 1], None)

            # tail: AV of the last head, its transposes, last projection
            for it in av_due.get(NSLOT, ()):
                it()
            for d in range(NSLOT, NSLOT + 3):
                for (tt, fb) in trans_due.get(d, ()):
                    emit_trans_fb(tt, fb)
            for ib in range(4):
                emit_proj(len(IQS) - 1, ib)

    nc.compile()
    return nc


def _make_in_maps(x, w_qkv, w_out):
    xt = np.ascontiguousarray(
        np.asarray(x, np.float32).transpose(0, 2, 1)).astype(np.float16)
    w_qkv = np.asarray(w_qkv, np.float32)
    w_out = np.asarray(w_out, np.float32)
    in_maps = []
    for c in range(NCORES):
        b, hg = divmod(c, HG)
        f0 = hg * HG_F
        in_maps.append({
            "xt": xt[b],
            "wq": np.ascontiguousarray(
                w_qkv[:, f0:f0 + HG_F]).astype(np.float16),
            "wk": np.ascontiguousarray(
                w_qkv[:, INNER + f0:INNER + f0 + HG_F]).astype(np.float16),
            "wv": np.ascontiguousarray(
                w_qkv[:, 2 * INNER + f0:2 * INNER + f0 + HG_F]).astype(
                    np.float16),
            "wo": np.ascontiguousarray(
                w_out[f0:f0 + HG_F, :]).astype(np.float16),
        })
    return in_maps


def run(x, w_qkv, w_out, **spmd_kwargs):
    """Build (once) + execute on 8 cores; returns BassKernelResults."""
    global _STATE
    from concourse.bass_utils import run_bass_kernel_spmd
    if _STATE is None:
        _STATE = _build_module()
    in_maps = _make_in_maps(x, w_qkv, w_out)
    return run_bass_kernel_spmd(_STATE, in_maps,
                                core_ids=list(range(NCORES)), **spmd_kwargs)


def kernel(x, w_qkv, w_out, b_out):
    res = run(x, w_qkv, w_out)
    parts = [np.asarray(res.results[c]["out"]) for c in range(NCORES)]
    b_out = np.asarray(b_out, np.float32)
    out = np.stack([parts[HG * b] + parts[HG * b + 1] for b in range(BATCH)])
    out += b_out[None, None, :]
    return out.astype(np.float32, copy=False)


# revision 60
# speedup vs baseline: 1.5059x; 1.0244x over previous
"""Trainium2 Bass kernel: multi-head attention (dense transformer block).

Sharding: 8 cores = 4 batches x 2 head-groups (8 heads each).

Per-core dataflow (all operands fp16; PSUM accumulation fp32):
    QT = Wq_hg^T @ x^T        [512 f, 2048 n]   (features on partitions)
    KT = Wk_hg^T @ x^T        [512 f, 2048 n]
    V  = x @ Wv_hg            [2048 j, 8 h, 64+1]  (ones column -> softmax Z)
    scores: per (head, j-block, i-chunk): S^T = K_h^T.T @ Q_h^T  [128 j, N i]
    exp on ScalarE over [128, 1024]-free granules -> P^T fp16 in SBUF
    AV (transposed): per (head, i-block): O_acc[128 i, 65] = sum_j P^T.T @ [V_h | 1]
    normalize on DVE: O = O_acc[:, :64] * (1/O_acc[:, 64])  -> O [n, f] fp16
    O^T via DMA-xbar transpose (SBUF->SBUF) -> OT [f, n]
    out = OT.T @ Wo  [2048 n, 1024 e] -> DRAM
Host: out[b] = partial[2b] + partial[2b+1] + b_out.

Schedule: ScalarE (33.5M exps/core, ~266us) is the bottleneck; everything
else is slotted around keeping it saturated. The i-range is processed in
three phases [1024, 512, 512]; the first is wide so the exp stream can hide
the QK/V projections, which are emitted as ~1.7us filler units placed just
before their consumer head needs them. AV of head h runs one slot behind its
exp; the output projection of each phase fills the next phase's PE slack.
"""

import os
import numpy as np

os.environ.setdefault("MYCRO_LOCAL_CACHE", "1")

DIM = 1024
HEADS = 16
DIM_HEAD = 64
INNER = HEADS * DIM_HEAD      # 1024
SEQ = 2048
BATCH = 4
NCORES = 8
HG = 2                        # tensor-parallel head groups
NH = HEADS // HG              # 8 heads per core
HG_F = NH * DIM_HEAD          # 512 local inner features
SCALE = DIM_HEAD ** -0.5      # 1/8
# fp16 Schraudolph exp on DVE: p = bitcast_fp16(int16(A*s_raw + B)).
# A = 2^10/ln2 * SCALE, B tuned against np.exp (bias cancels in softmax).
EXP_A = 1024.0 / 0.6931471805599453 * SCALE
EXP_B = 15300.0

DCS = DIM // 128              # 8 contraction chunks for projections
NTS = SEQ // 512              # 4 n-chunks for QK-proj
FBS = HG_F // 128             # 4 feature blocks of QT/KT (2 heads each)
JBS = SEQ // 128              # 16 key blocks
EBS = DIM // 512              # 2 output column chunks

# i-range phases: (i0, width). Phase 0 is wide so the exp stream can hide
# the projection prologue; the last phases are narrow so the serial tail
# (AV of the last head -> transpose -> projection -> DMA) is short.
IQS = [(0, 1024), (1024, 512), (1536, 512)]

_STATE = None


def _build_module():
    from contextlib import ExitStack
    import concourse.bacc as bacc
    import concourse.tile as tile
    import concourse.mybir as mybir

    f32 = mybir.dt.float32
    f16 = mybir.dt.float16
    Exp = mybir.ActivationFunctionType.Exp
    Copy = mybir.ActivationFunctionType.Copy

    nc = bacc.Bacc("TRN2", target_bir_lowering=False, debug=False,
                   num_devices=NCORES)

    xt_d = nc.dram_tensor("xt", [DIM, SEQ], f16, kind="ExternalInput").ap()
    wq_d = nc.dram_tensor("wq", [DIM, HG_F], f16, kind="ExternalInput").ap()
    wk_d = nc.dram_tensor("wk", [DIM, HG_F], f16, kind="ExternalInput").ap()
    wv_d = nc.dram_tensor("wv", [DIM, HG_F], f16, kind="ExternalInput").ap()
    wo_d = nc.dram_tensor("wo", [HG_F, DIM], f16, kind="ExternalInput").ap()
    out_d = nc.dram_tensor("out", [SEQ, DIM], f32, kind="ExternalOutput").ap()

    with tile.TileContext(nc) as tc, ExitStack() as ctx:
        # --- PSUM pools: 2x2 (scores) + 2 (AV) + 2 (proj/qkv) = 8 banks ---
        ps_s_pool = ctx.enter_context(
            tc.tile_pool(name="pss", bufs=2, space="PSUM"))
        ps_av_pool = ctx.enter_context(
            tc.tile_pool(name="psav", bufs=2, space="PSUM"))
        ps_acc_pool = ctx.enter_context(
            tc.tile_pool(name="psacc", bufs=2, space="PSUM"))

        # SBUF tile slots are 4KB-per-partition quanta; small tiles are
        # packed into 4KB-aligned groups to avoid 4x padding waste.
        qt_pool = ctx.enter_context(tc.tile_pool(name="qtp", bufs=FBS))
        kt_pool = ctx.enter_context(tc.tile_pool(name="ktp", bufs=FBS))
        v_pool = ctx.enter_context(tc.tile_pool(name="vpp", bufs=1))
        pt_pool = ctx.enter_context(tc.tile_pool(name="ptp", bufs=12))
        rc_pool = ctx.enter_context(tc.tile_pool(name="rcp", bufs=1))
        o_pool = ctx.enter_context(tc.tile_pool(name="opp", bufs=4))
        ot_pool = ctx.enter_context(tc.tile_pool(name="otp", bufs=FBS))
        wo_pool = ctx.enter_context(tc.tile_pool(name="wop", bufs=2))
        outs_pool = ctx.enter_context(tc.tile_pool(name="outs", bufs=2))

        qt = [qt_pool.tile([128, SEQ], f16, tag="qt", name=f"qt{i}")
              for i in range(FBS)]
        kt = [kt_pool.tile([128, SEQ], f16, tag="kt", name=f"kt{i}")
              for i in range(FBS)]
        vpt = v_pool.tile([128, JBS, NH, DIM_HEAD + 1], f16, tag="vp",
                          name="vpt")
        obt = [o_pool.tile([128, 4, HG_F], f16, tag="ob", name=f"ob{i}")
               for i in range(4)]

        def ob_ap(nbg):
            return obt[nbg // 4][:, nbg % 4, :]
        ot = [ot_pool.tile([128, SEQ], f16, tag="ot", name=f"ot{i}")
              for i in range(FBS)]
        wot = [wo_pool.tile([128, 2, DIM], f16, tag="wo", name=f"wo{i}")
               for i in range(2)]
        rct = rc_pool.tile([128, 16], f32, tag="rc", name="rct")
        nc.vector.memset(vpt[:, :, :, DIM_HEAD:DIM_HEAD + 1], 1.0)

        with tc.tile_pool(name="xtp", bufs=DCS // 2) as xt_pool, \
             tc.tile_pool(name="wqp", bufs=2) as wq_pool, \
             tc.tile_pool(name="wkp", bufs=2) as wk_pool, \
             tc.tile_pool(name="wvp", bufs=2) as wv_pool:

            # xt split into two n-half tile-packs per pair of 128-row
            # chunks so the first projection matmuls can start after ~half
            # the input DMA. [128, 2, 1024]: dim1 = dc parity.
            xtp = [[xt_pool.tile([128, 2, 1024], f16, tag=f"xt{hf}",
                                 name=f"xt{p}_{hf}") for p in range(DCS // 2)]
                   for hf in range(2)]
            wvp = [wv_pool.tile([128, 4, HG_F], f16, tag="wv",
                                name=f"wv{p}") for p in range(2)]
            wqp = [wq_pool.tile([128, 4, HG_F], f16, tag="wq",
                                name=f"wq{p}") for p in range(2)]
            wkp = [wk_pool.tile([128, 4, HG_F], f16, tag="wk",
                                name=f"wk{p}") for p in range(2)]
            for dc in range(4):
                nc.sync.dma_start(wvp[0][:, dc, :],
                                  wv_d[dc * 128:(dc + 1) * 128, :])
            for dc in range(2):
                nc.sync.dma_start(xtp[0][dc // 2][:, dc % 2, :],
                                  xt_d[dc * 128:(dc + 1) * 128, 0:1024])
            for dc in range(4, DCS):
                nc.sync.dma_start(wvp[1][:, dc - 4, :],
                                  wv_d[dc * 128:(dc + 1) * 128, :])
            for dc in range(2, DCS):
                nc.sync.dma_start(xtp[0][dc // 2][:, dc % 2, :],
                                  xt_d[dc * 128:(dc + 1) * 128, 0:1024])
            for dc in range(DCS):
                nc.sync.dma_start(xtp[1][dc // 2][:, dc % 2, :],
                                  xt_d[dc * 128:(dc + 1) * 128, 1024:2048])
            for dc in range(DCS):
                nc.sync.dma_start(wkp[dc // 4][:, dc % 4, :],
                                  wk_d[dc * 128:(dc + 1) * 128, :])
            for dc in range(DCS):
                nc.sync.dma_start(wqp[dc // 4][:, dc % 4, :],
                                  wq_d[dc * 128:(dc + 1) * 128, :])
            for fc in range(FBS):
                nc.sync.dma_start(wot[fc // 2][:, fc % 2, :],
                                  wo_d[fc * 128:(fc + 1) * 128, :])

            def xt_ap(dc, n0, n1):
                hf = n0 // 1024
                assert n1 <= (hf + 1) * 1024
                return xtp[hf][dc // 2][:, dc % 2,
                                        n0 - hf * 1024:n1 - hf * 1024]

            def emit_v(nb):
                ps = ps_acc_pool.tile([128, NH, DIM_HEAD], f32, tag="acc")
                for dc in range(DCS):
                    nc.tensor.matmul(
                        ps[:, :, :],
                        xt_ap(dc, nb * 128, (nb + 1) * 128),
                        wvp[dc // 4][:, dc % 4, :],
                        start=(dc == 0), stop=(dc == DCS - 1))
                nc.vector.tensor_copy(vpt[:, nb, :, 0:DIM_HEAD], ps[:, :, :])

            def emit_fb_nt(fb, w, dst, nt, on_dve=False):
                # one 512-n chunk of one projection f-block of Q or K
                ps = ps_acc_pool.tile([128, 512], f32, tag="acc")
                for dc in range(DCS):
                    nc.tensor.matmul(
                        ps[:, :],
                        w[dc // 4][:, dc % 4, fb * 128:(fb + 1) * 128],
                        xt_ap(dc, nt * 512, (nt + 1) * 512),
                        start=(dc == 0), stop=(dc == DCS - 1))
                dst_ap = dst[fb][:, nt * 512:(nt + 1) * 512]
                if on_dve:
                    nc.vector.tensor_copy(dst_ap, ps[:, :])
                else:
                    nc.scalar.activation(dst_ap, ps[:, :], Copy)

            # ---- prologue: V (AV of head 0 needs all of it one slot into
            # the attention stream), kt[0] and the first two qt[0] chunks ----
            for nb in range(12):
                emit_v(nb)
            emit_fb_nt(0, wkp, kt, 0)
            emit_fb_nt(0, wkp, kt, 1)
            emit_fb_nt(0, wqp, qt, 0)
            emit_fb_nt(0, wqp, qt, 1)
            emit_fb_nt(0, wkp, kt, 2)
            emit_fb_nt(0, wkp, kt, 3)

            # ---- attention, software-pipelined over (phase, head) ----
            pt_tiles = {}   # (t, h) -> list of granule tiles

            def emit_scores_granule(t, h, g, dve=False):
                i0, w = IQS[t]
                fb, p0 = h // 2, (h % 2) * 64
                packs = pt_tiles[(t, h)]
                if w == 1024:
                    jb = g
                    ps = ps_s_pool.tile([128, 1, 1024], f32, tag="sc")
                    for ic in range(2):
                        nc.tensor.matmul(
                            ps[:, 0, ic * 512:(ic + 1) * 512],
                            kt[fb][p0:p0 + 64, jb * 128:(jb + 1) * 128],
                            qt[fb][p0:p0 + 64,
                                   i0 + ic * 512:i0 + (ic + 1) * 512],
                            start=True, stop=True)
                    if g % 2 == 0:
                        packs.append(pt_pool.tile(
                            [128, 2, 1024], f16, tag="pt",
                            name=f"pt{t}_{h}_{g // 2}"))
                    dst = packs[-1][:, g % 2:g % 2 + 1, :]
                else:
                    jpg = 1024 // w          # j-blocks per granule (2 or 4)
                    ps = ps_s_pool.tile([128, jpg, w], f32, tag="sc")
                    for j2 in range(jpg):
                        jb = g * jpg + j2
                        nc.tensor.matmul(
                            ps[:, j2, :],
                            kt[fb][p0:p0 + 64, jb * 128:(jb + 1) * 128],
                            qt[fb][p0:p0 + 64, i0:i0 + w],
                            start=True, stop=True)
                    if g % 2 == 0:
                        packs.append(pt_pool.tile(
                            [128, 2 * jpg, w], f16, tag="pt",
                            name=f"pt{t}_{h}_{g // 2}"))
                    dst = packs[-1][:, (g % 2) * jpg:(g % 2 + 1) * jpg, :]
                if dve:
                    import concourse.mybir as mb
                    nc.vector.tensor_scalar(
                        dst.bitcast(mb.dt.int16), ps[:, :, :],
                        EXP_A, EXP_B,
                        mb.AluOpType.mult, mb.AluOpType.add)
                else:
                    nc.scalar.activation(dst, ps[:, :, :], Exp, scale=SCALE)

            av_n = [0]

            def emit_av(t, h, ib):
                i0, w = IQS[t]
                pts = pt_tiles[(t, h)]
                acc = ps_av_pool.tile([128, DIM_HEAD + 1], f32, tag="av")
                jpp = max(2, 2048 // w)      # j-blocks per pt pack
                for jb in range(JBS):
                    lhs = pts[jb // jpp][:, jb % jpp,
                              ib * 128:(ib + 1) * 128]
                    nc.tensor.matmul(
                        acc[:, :], lhs, vpt[:, jb, h, :],
                        start=(jb == 0), stop=(jb == JBS - 1))
                rc = rct[:, av_n[0] % 16:av_n[0] % 16 + 1]
                av_n[0] += 1
                with nc.allow_low_precision("softmax recip"):
                    nc.vector.reciprocal(rc, acc[:, DIM_HEAD:DIM_HEAD + 1])
                nbg = i0 // 128 + ib
                nc.vector.tensor_scalar(
                    ob_ap(nbg)[:, h * DIM_HEAD:(h + 1) * DIM_HEAD],
                    acc[:, 0:DIM_HEAD], rc, None,
                    mybir.AluOpType.mult)

            def emit_trans_fb(t, fb):
                i0, w = IQS[t]
                for ib in range(w // 128):
                    nbg = i0 // 128 + ib
                    nc.sync.dma_start_transpose(
                        ot[fb][:, nbg * 128:(nbg + 1) * 128],
                        ob_ap(nbg)[:, fb * 128:(fb + 1) * 128])

            def emit_trans(t):
                for fb in range(FBS):
                    emit_trans_fb(t, fb)

            def emit_proj(t, ib):
                nbg = IQS[t][0] // 128 + ib
                os_ = outs_pool.tile([128, EBS, 512], f32, tag="os")
                for eb in range(EBS):
                    ps = ps_acc_pool.tile([128, 512], f32, tag="acc")
                    for fc in range(FBS):
                        nc.tensor.matmul(
                            ps[:, :],
                            ot[fc][:, nbg * 128:(nbg + 1) * 128],
                            wot[fc // 2][:, fc % 2,
                                         eb * 512:(eb + 1) * 512],
                            start=(fc == 0), stop=(fc == FBS - 1))
                    nc.vector.tensor_copy(os_[:, eb, :], ps[:, :])
                    nc.sync.dma_start(
                        out_d[nbg * 128:(nbg + 1) * 128,
                              eb * 512:(eb + 1) * 512],
                        os_[:, eb, :])

            def u_fb(fb, w, dst, nt):
                return lambda: emit_fb_nt(fb, w, dst, nt, on_dve=True)

            def u_av(i, ib):
                return lambda: emit_av(seq[i][0], seq[i][1], ib)

            def u_v(nb):
                return lambda: emit_v(nb)

            def u_proj(t, ib):
                return lambda: emit_proj(t, ib)

            seq = [(t, h) for t in range(len(IQS)) for h in range(NH)]
            NSLOT = len(seq)

            # AV of head-slot i runs one slot later (pt pool holds two head
            # windows of the wide phase).
            av_due = {}
            for i in range(NSLOT):
                av_due.setdefault(i + 1, []).extend(
                    u_av(i, ib) for ib in range(IQS[seq[i][0]][1] // 128))

            # Projection filler units, placed just before their consumer.
            fill = {
                (0, 0): [u_v(12), u_v(13), u_v(14), u_v(15)],
                (0, 1): [u_fb(1, wkp, kt, 0), u_fb(1, wkp, kt, 1),
                         u_fb(1, wkp, kt, 2), u_fb(1, wkp, kt, 3),
                         u_fb(1, wqp, qt, 0), u_fb(1, wqp, qt, 1)],
                (0, 2): [u_fb(2, wkp, kt, 0), u_fb(2, wkp, kt, 1)],
                (0, 3): [u_fb(2, wkp, kt, 2), u_fb(2, wkp, kt, 3),
                         u_fb(2, wqp, qt, 0), u_fb(2, wqp, qt, 1)],
                (0, 4): [u_fb(3, wkp, kt, 0), u_fb(3, wkp, kt, 1)],
                (0, 5): [u_fb(3, wkp, kt, 2), u_fb(3, wkp, kt, 3),
                         u_fb(3, wqp, qt, 0), u_fb(3, wqp, qt, 1)],
                (0, 6): [u_fb(0, wqp, qt, 2), u_fb(0, wqp, qt, 3)],
                (0, 7): [u_fb(1, wqp, qt, 2), u_fb(1, wqp, qt, 3)],
                (1, 0): [u_fb(2, wqp, qt, 2), u_fb(2, wqp, qt, 3)],
                (1, 1): [u_fb(3, wqp, qt, 2), u_fb(3, wqp, qt, 3)],
            }
            # output projection of phase t fills slots of phase t+1 (after
            # its transposes, which follow the AV of each head pair)
            for h in range(2, NH):
                fill.setdefault((1, h), []).append(u_proj(0, h - 2))
            fill.setdefault((2, 0), []).append(u_proj(0, 6))
            fill.setdefault((2, 1), []).append(u_proj(0, 7))
            for ib in range(4):
                fill.setdefault((2, 2 + ib), []).append(u_proj(1, ib))

            # transpose of O columns for head pair (2fb, 2fb+1) of phase t
            # becomes ready one slot after AV(t, 2fb+1), i.e. at slot index
            # t*NH + 2fb + 3; the fb=3 pair rolls into the next phase.
            trans_due = {}
            for t in range(len(IQS)):
                for fb in range(FBS):
                    trans_due.setdefault(t * NH + 2 * fb + 3, []).append(
                        (t, fb))

            for idx, (t, h) in enumerate(seq):
                pt_tiles[(t, h)] = []
                ngr = IQS[t][1] // 64
                for (tt, fb) in trans_due.get(idx, ()):
                    emit_trans_fb(tt, fb)   # DMA-engine work, no PE cost
                items = av_due.get(idx, []) + list(fill.get((t, h), ()))
                for g in range(ngr):
                    emit_scores_granule(t, h, g,
                                        dve=(t >= 1 and g in (2, 6)))
                    if g < len(items):
                        items[g]()
                for it in items[ngr:]:
                    it()
                if idx >= 1:
                    pt_tiles.pop(seq[idx - 1], None)

            # tail: pipeline the last head's AV chunks with the final
            # per-block transpose and output projection
            tl = len(IQS) - 1
            for ib in range(IQS[tl][1] // 128):
                emit_av(tl, NH - 1, ib)
                nbg = IQS[tl][0] // 128 + ib
                nc.sync.dma_start_transpose(
                    ot[FBS - 1][:, nbg * 128:(nbg + 1) * 128],
                    ob_ap(nbg)[:, (FBS - 1) * 128:FBS * 128])
                emit_proj(tl, ib)

    nc.compile()
    return nc


def _make_in_maps(x, w_qkv, w_out):
    xt = np.ascontiguousarray(
        np.asarray(x, np.float32).transpose(0, 2, 1)).astype(np.float16)
    w_qkv = np.asarray(w_qkv, np.float32)
    w_out = np.asarray(w_out, np.float32)
    in_maps = []
    for c in range(NCORES):
        b, hg = divmod(c, HG)
        f0 = hg * HG_F
        in_maps.append({
            "xt": xt[b],
            "wq": np.ascontiguousarray(
                w_qkv[:, f0:f0 + HG_F]).astype(np.float16),
            "wk": np.ascontiguousarray(
                w_qkv[:, INNER + f0:INNER + f0 + HG_F]).astype(np.float16),
            "wv": np.ascontiguousarray(
                w_qkv[:, 2 * INNER + f0:2 * INNER + f0 + HG_F]).astype(
                    np.float16),
            "wo": np.ascontiguousarray(
                w_out[f0:f0 + HG_F, :]).astype(np.float16),
        })
    return in_maps


def run(x, w_qkv, w_out, **spmd_kwargs):
    """Build (once) + execute on 8 cores; returns BassKernelResults."""
    global _STATE
    from concourse.bass_utils import run_bass_kernel_spmd
    if _STATE is None:
        _STATE = _build_module()
    in_maps = _make_in_maps(x, w_qkv, w_out)
    return run_bass_kernel_spmd(_STATE, in_maps,
                                core_ids=list(range(NCORES)), **spmd_kwargs)


def kernel(x, w_qkv, w_out, b_out):
    res = run(x, w_qkv, w_out)
    parts = [np.asarray(res.results[c]["out"]) for c in range(NCORES)]
    b_out = np.asarray(b_out, np.float32)
    out = np.stack([parts[HG * b] + parts[HG * b + 1] for b in range(BATCH)])
    out += b_out[None, None, :]
    return out.astype(np.float32, copy=False)
